# revision 7
# baseline (speedup 1.0000x reference)
"""Trainium2 Bass kernel for causal multi-head attention with RoPE.

Problem shapes (hardcoded): x [2,2048,2048] f32, Wqkv [6144,2048], Wout [2048,2048],
cos/sin [2048,128]. 16 heads x 128 head-dim.

Sharding: tensor-parallel over heads -- 2 heads per core on 8 cores.
Each core computes qkv projection for its heads, RoPE, causal SDPA, and its
slice of the output projection (row-parallel); host sums the 8 partials.

All on-device layouts keep tokens on the free dimension ([dim, tokens]) so no
transposes are ever needed:
  - Q/K produced as qT/kT [hd, tok] directly from the projection.
  - V produced as v [tok, hd] (other matmul orientation).
  - scoresT [k_tok, q_tok] = kT_tile.T @ qT -> softmax over the partition dim:
    no max subtraction (scores are provably bounded ~N(0,1)), l = column sums
    via a ones-vector matmul, normalization applied to the attention output.
  - attention output oT [hd, q_tok] = v_chunk.T @ probsT, accumulated in PSUM.
  - output projection outT [D, tok] = WoutT_chunk.T @ oT.
RoPE rotate_half is a fixed +-1 permutation => done with a 128x128 matmul.
"""

import numpy as np
import ml_dtypes

B, T, D, H = 2, 2048, 2048, 16
HD = 128
N_CORES = 8
HPC = H // N_CORES          # heads per core = 2
TOK = B * T                 # 4096 flattened tokens
NT = TOK // 512             # 8 token tiles of 512
KC = D // 128               # 16 contraction chunks for the projections
SCALE = 1.0 / float(np.sqrt(HD))

BF16 = ml_dtypes.bfloat16

_CACHE = {}


def _emit_body(nc, tc, pools, io):
    """Emit one full forward pass."""
    import concourse.bass as bass  # noqa: F401
    import concourse.mybir as mybir

    dt = mybir.dt
    consts, xpool, big, temps, probsp, outp, psum, psum_o, psum_l = pools
    xt_d, wq_d, wk_d, wv_d, wo_d, cs_d, sn_d, pt_d, onec_d, oner_d, msk_d, out_d = io

    # ---- resident constants / weights in SBUF ----
    wq_sb = consts.tile([128, KC, 256], dt.bfloat16, tag="wq")
    wk_sb = consts.tile([128, KC, 256], dt.bfloat16, tag="wk")
    wv_sb = consts.tile([128, KC, 256], dt.bfloat16, tag="wv")
    nc.sync.dma_start(out=wq_sb, in_=wq_d.rearrange("(c p) e -> p c e", p=128))
    nc.sync.dma_start(out=wk_sb, in_=wk_d.rearrange("(c p) e -> p c e", p=128))
    nc.sync.dma_start(out=wv_sb, in_=wv_d.rearrange("(c p) e -> p c e", p=128))
    wo_sb = consts.tile([128, HPC, D], dt.bfloat16, tag="wo")
    nc.sync.dma_start(out=wo_sb, in_=wo_d.rearrange("(h p) e -> p h e", p=128))
    cs_sb = consts.tile([128, TOK], dt.bfloat16, tag="cs")
    sn_sb = consts.tile([128, TOK], dt.bfloat16, tag="sn")
    nc.sync.dma_start(out=cs_sb, in_=cs_d[:])
    nc.sync.dma_start(out=sn_sb, in_=sn_d[:])
    pt_sb = consts.tile([128, 128], dt.bfloat16, tag="pt")
    nc.sync.dma_start(out=pt_sb, in_=pt_d[:])
    onec_sb = consts.tile([128, 1], dt.bfloat16, tag="onec")
    nc.sync.dma_start(out=onec_sb, in_=onec_d[:])
    oner_sb = consts.tile([1, 128], dt.float32, tag="oner")
    nc.sync.dma_start(out=oner_sb, in_=oner_d[:])
    msk_sb = consts.tile([128, 4, 512], dt.bfloat16, tag="msk")
    nc.sync.dma_start(out=msk_sb, in_=msk_d.rearrange("m p t -> p m t"))

    # ---- resident activations ----
    qT = big.tile([128, HPC, TOK], dt.bfloat16, tag="qT")   # [hd, head, tok]
    kT = big.tile([128, HPC, TOK], dt.bfloat16, tag="kT")
    v = big.tile([128, TOK // 128, 256], dt.bfloat16, tag="v")   # [tok%128, tok chunk, head*hd]
    oTn = big.tile([128, HPC, TOK], dt.bfloat16, tag="oTn")  # normalized attn out

    xt_r = xt_d.rearrange("(c p) t -> p c t", p=128)  # [128, 16, 4096]

    # ================= Phase A: QKV projection + RoPE =================
    for tt in range(NT):
        t0 = tt * 512
        xt_sb = xpool.tile([128, KC, 512], dt.bfloat16, tag="xt")
        nc.sync.dma_start(out=xt_sb, in_=xt_r[:, :, t0:t0 + 512])

        # Q and K -> [hd, tok] with RoPE
        for which, w_sb, dst in (("q", wq_sb, qT), ("k", wk_sb, kT)):
            for m in range(HPC):  # head
                ps = psum.tile([128, 512], dt.float32, tag="mm512")
                for kc in range(KC):
                    nc.tensor.matmul(
                        ps,
                        lhsT=w_sb[:, kc, m * 128:(m + 1) * 128],
                        rhs=xt_sb[:, kc, :],
                        start=(kc == 0),
                        stop=(kc == KC - 1),
                    )
                raw = temps.tile([128, 512], dt.bfloat16, tag="raw")
                nc.scalar.copy(out=raw, in_=ps)
                # rotate_half via permutation matmul
                psr = psum.tile([128, 512], dt.float32, tag="mm512")
                nc.tensor.matmul(psr, lhsT=pt_sb, rhs=raw, start=True, stop=True)
                t1 = temps.tile([128, 512], dt.bfloat16, tag="t1")
                nc.vector.tensor_tensor(t1, raw, cs_sb[:, t0:t0 + 512], mybir.AluOpType.mult)
                t2 = temps.tile([128, 512], dt.bfloat16, tag="t2")
                nc.vector.tensor_tensor(t2, psr, sn_sb[:, t0:t0 + 512], mybir.AluOpType.mult)
                nc.vector.tensor_tensor(dst[:, m, t0:t0 + 512], t1, t2, mybir.AluOpType.add)

        # V -> [tok, hd] orientation
        for sub in range(4):
            psv = psum.tile([128, 256], dt.float32, tag="mm512")
            for kc in range(KC):
                nc.tensor.matmul(
                    psv,
                    lhsT=xt_sb[:, kc, sub * 128:(sub + 1) * 128],
                    rhs=wv_sb[:, kc, :],
                    start=(kc == 0),
                    stop=(kc == KC - 1),
                )
            nc.scalar.copy(out=v[:, tt * 4 + sub, :], in_=psv)

    # ================= Phase B: causal SDPA per (head, batch) =================
    for h in range(HPC):
        for b in range(B):
            tb = b * T  # token base of this batch
            for qb in range(4):  # q blocks of 512 within the batch
                q0 = tb + qb * 512
                nk = 4 * (qb + 1)  # causal: k chunks of 128 up to q block end
                ps_o = psum_o.tile([128, 512], dt.float32, tag="acc_o")
                ps_l = psum_l.tile([1, 512], dt.float32, tag="acc_l")
                for kc in range(nk):
                    ps_s = psum.tile([128, 512], dt.float32, tag="mm512")
                    nc.tensor.matmul(
                        ps_s,
                        lhsT=kT[:, h, tb + kc * 128: tb + (kc + 1) * 128],
                        rhs=qT[:, h, q0:q0 + 512],
                        start=True,
                        stop=True,
                    )
                    pr = probsp.tile([128, 512], dt.bfloat16, tag="probs")
                    nc.scalar.activation(pr, ps_s, mybir.ActivationFunctionType.Exp,
                                         scale=SCALE)
                    moff = kc - 4 * qb
                    if moff >= 0:  # diagonal tile: causal mask
                        nc.vector.tensor_tensor(pr, pr, msk_sb[:, moff, :],
                                                mybir.AluOpType.mult)
                    nc.tensor.matmul(ps_l, lhsT=onec_sb, rhs=pr,
                                     start=(kc == 0), stop=(kc == nk - 1))
                    nc.tensor.matmul(
                        ps_o,
                        lhsT=v[:, b * 16 + kc, h * 128:(h + 1) * 128],
                        rhs=pr,
                        start=(kc == 0),
                        stop=(kc == nk - 1),
                    )
                # normalize: oTn = ps_o * (1 / broadcast(l))
                l_row = temps.tile([1, 512], dt.float32, tag="l_row")
                nc.scalar.copy(out=l_row, in_=ps_l)
                ps_b = psum.tile([128, 512], dt.float32, tag="mm512")
                nc.tensor.matmul(ps_b, lhsT=oner_sb, rhs=l_row, start=True, stop=True)
                rb = temps.tile([128, 512], dt.float32, tag="rb")
                nc.vector.reciprocal(rb, ps_b)
                nc.vector.tensor_tensor(oTn[:, h, q0:q0 + 512], ps_o, rb,
                                        mybir.AluOpType.mult)

    # ================= Phase C: output projection (row-parallel slice) =========
    for tt in range(NT):
        t0 = tt * 512
        for dtile in range(D // 128):
            ps = psum.tile([128, 512], dt.float32, tag="mm512")
            for h in range(HPC):
                nc.tensor.matmul(
                    ps,
                    lhsT=wo_sb[:, h, dtile * 128:(dtile + 1) * 128],
                    rhs=oTn[:, h, t0:t0 + 512],
                    start=(h == 0),
                    stop=(h == HPC - 1),
                )
            o_sb = outp.tile([128, 512], dt.float32, tag="o_sb")
            if dtile % 2 == 0:
                nc.scalar.copy(out=o_sb, in_=ps)
            else:
                nc.vector.tensor_copy(out=o_sb, in_=ps)
            nc.sync.dma_start(out=out_d[dtile * 128:(dtile + 1) * 128, t0:t0 + 512],
                              in_=o_sb)


def build_nc(reps=1):
    """Build the Bass module. reps>1 wraps the body in a For_i loop executing
    it that many times (used only for wall-clock timing measurements)."""
    import concourse.bass as bass
    import concourse.mybir as mybir
    import concourse.tile as tile

    dt = mybir.dt
    nc = bass.Bass("TRN2", target_bir_lowering=False, debug=False,
                   num_devices=N_CORES)

    xt_d = nc.dram_tensor("xt", [D, TOK], dt.bfloat16, kind="ExternalInput")
    wq_d = nc.dram_tensor("wq", [D, 256], dt.bfloat16, kind="ExternalInput")
    wk_d = nc.dram_tensor("wk", [D, 256], dt.bfloat16, kind="ExternalInput")
    wv_d = nc.dram_tensor("wv", [D, 256], dt.bfloat16, kind="ExternalInput")
    wo_d = nc.dram_tensor("wo", [256, D], dt.bfloat16, kind="ExternalInput")
    cs_d = nc.dram_tensor("cs", [128, TOK], dt.bfloat16, kind="ExternalInput")
    sn_d = nc.dram_tensor("sn", [128, TOK], dt.bfloat16, kind="ExternalInput")
    pt_d = nc.dram_tensor("pt", [128, 128], dt.bfloat16, kind="ExternalInput")
    onec_d = nc.dram_tensor("onec", [128, 1], dt.bfloat16, kind="ExternalInput")
    oner_d = nc.dram_tensor("oner", [1, 128], dt.float32, kind="ExternalInput")
    msk_d = nc.dram_tensor("msk", [4, 128, 512], dt.bfloat16, kind="ExternalInput")
    out_d = nc.dram_tensor("out", [D, TOK], dt.float32, kind="ExternalOutput")
    io = (xt_d, wq_d, wk_d, wv_d, wo_d, cs_d, sn_d, pt_d, onec_d, oner_d,
          msk_d, out_d)

    with tile.TileContext(nc) as tc:
        import contextlib
        with contextlib.ExitStack() as ctx:
            consts = ctx.enter_context(tc.tile_pool(name="consts", bufs=1))
            xpool = ctx.enter_context(tc.tile_pool(name="xpool", bufs=2))
            big = ctx.enter_context(tc.tile_pool(name="big", bufs=1))
            temps = ctx.enter_context(tc.tile_pool(name="temps", bufs=4))
            probsp = ctx.enter_context(tc.tile_pool(name="probs", bufs=3))
            outp = ctx.enter_context(tc.tile_pool(name="outp", bufs=4))
            psum = ctx.enter_context(tc.tile_pool(name="psum", bufs=3,
                                                  space="PSUM"))
            psum_o = ctx.enter_context(tc.tile_pool(name="psum_o", bufs=2,
                                                    space="PSUM"))
            psum_l = ctx.enter_context(tc.tile_pool(name="psum_l", bufs=2,
                                                    space="PSUM"))
            pools = (consts, xpool, big, temps, probsp, outp, psum, psum_o,
                     psum_l)
            if reps > 1:
                with tc.For_i(0, reps, 1):
                    _emit_body(nc, tc, pools, io)
            else:
                _emit_body(nc, tc, pools, io)

    return nc


def _split_sync_waits(nc, max_waits=1):
    """Walrus in this env rejects instructions with too many sync waits.
    Hoist excess waits onto preceding same-engine nops."""
    import bass_rust
    import concourse.mybir as mybir

    n_split = 0
    for f in nc.m.functions:
        for bb in f.blocks:
            insts = bb.instructions
            new = []
            dirty = False
            for inst in insts:
                si = inst.sync_info
                if si is not None and si.on_wait and len(si.on_wait) > max_waits:
                    waits = list(si.on_wait)
                    for j, w in enumerate(waits[:-max_waits]):
                        n = mybir.InstNoOp(name=f"{inst.name}-wsplit{j}",
                                           ins=[], outs=[])
                        n.engine = inst.engine
                        n.sync_info = bass_rust.SyncInfo(on_wait=[w], on_update=[])
                        new.append(n)
                        n_split += 1
                    si.on_wait = waits[-max_waits:]
                    inst.sync_info = si
                    dirty = True
                new.append(inst)
            if dirty:
                bb.instructions = new
    return n_split


def _host_prep(x, cos, sin, Wqkv, Wout):
    """Shard + lay out inputs for each core. Returns in_maps list."""
    xf = np.ascontiguousarray(x.reshape(TOK, D).T).astype(BF16)        # [D, TOK]
    csT = np.ascontiguousarray(cos.T)                                   # [128, T]
    snT = np.ascontiguousarray(sin.T)
    cs4 = np.concatenate([csT] * B, axis=1).astype(BF16)                # [128, TOK]
    sn4 = np.concatenate([snT] * B, axis=1).astype(BF16)

    # rotate_half permutation: rot = P @ u ; pt = P.T
    P = np.zeros((128, 128), dtype=np.float32)
    P[np.arange(64), np.arange(64) + 64] = -1.0
    P[np.arange(64) + 64, np.arange(64)] = 1.0
    pt = np.ascontiguousarray(P.T).astype(BF16)

    onec = np.ones((128, 1), dtype=np.float32).astype(BF16)
    oner = np.ones((1, 128), dtype=np.float32)

    msk = np.zeros((4, 128, 512), dtype=np.float32)
    for m in range(4):
        off = m * 128
        kk = np.arange(128)[:, None]
        qq = np.arange(512)[None, :]
        msk[m] = (off + kk <= qq).astype(np.float32)
    msk = msk.astype(BF16)

    in_maps = []
    for c in range(N_CORES):
        r0 = c * HPC * HD
        r1 = (c + 1) * HPC * HD
        wq = np.ascontiguousarray(Wqkv[r0:r1, :].T).astype(BF16)          # [D, 256]
        wk = np.ascontiguousarray(Wqkv[D + r0:D + r1, :].T).astype(BF16)
        wv = np.ascontiguousarray(Wqkv[2 * D + r0:2 * D + r1, :].T).astype(BF16)
        wo = np.ascontiguousarray(Wout[:, r0:r1].T).astype(BF16)          # [256, D]
        in_maps.append({
            "xt": xf, "wq": wq, "wk": wk, "wv": wv, "wo": wo,
            "cs": cs4, "sn": sn4, "pt": pt, "onec": onec, "oner": oner,
            "msk": msk,
        })
    return in_maps


def kernel(x, cos, sin, Wqkv, Wout):
    from concourse.bass_utils import run_bass_kernel_spmd

    x = np.asarray(x, dtype=np.float32)
    cos = np.asarray(cos, dtype=np.float32)
    sin = np.asarray(sin, dtype=np.float32)
    Wqkv = np.asarray(Wqkv, dtype=np.float32)
    Wout = np.asarray(Wout, dtype=np.float32)

    if "nc" not in _CACHE:
        nc = build_nc()
        _split_sync_waits(nc, max_waits=1)
        _CACHE["nc"] = nc
    nc = _CACHE["nc"]

    in_maps = _host_prep(x, cos, sin, Wqkv, Wout)
    res = run_bass_kernel_spmd(nc, in_maps, core_ids=list(range(N_CORES)))
    acc = np.zeros((D, TOK), dtype=np.float32)
    for c in range(N_CORES):
        acc += res.results[c]["out"]
    return np.ascontiguousarray(acc.T).reshape(B, T, D)


# revision 17
# speedup vs baseline: 13.7176x; 13.7176x over previous
"""Trainium2 Bass kernel for causal multi-head attention with RoPE.

Problem shapes (hardcoded): x [2,2048,2048] f32, Wqkv [6144,2048], Wout [2048,2048],
cos/sin [2048,128]. 16 heads x 128 head-dim.

Sharding: tensor-parallel over heads -- 2 heads per core on 8 cores.
Each core computes qkv projection for its heads, RoPE, causal SDPA, and its
slice of the output projection (row-parallel); host sums the 8 partials.

All on-device layouts keep tokens on the free dimension ([dim, tokens]) so no
transposes are ever needed:
  - Q/K produced as qT/kT [hd, tok] directly from the projection.
  - V produced as v [tok, hd] (other matmul orientation).
  - scoresT [k_tok, q_tok] = kT_tile.T @ qT -> softmax over the partition dim:
    no max subtraction (scores are provably bounded ~N(0,1)), l = column sums
    via a ones-vector matmul, normalization applied to the attention output.
  - attention output oT [hd, q_tok] = v_chunk.T @ probsT, accumulated in PSUM.
  - output projection outT [D, tok] = WoutT_chunk.T @ oT.
RoPE rotate_half is a fixed +-1 permutation => done with a 128x128 matmul.
"""

import numpy as np
import ml_dtypes

B, T, D, H = 2, 2048, 2048, 16
HD = 128
N_CORES = 8
HPC = H // N_CORES          # heads per core = 2
TOK = B * T                 # 4096 flattened tokens
NT = TOK // 512             # 8 token tiles of 512
KC = D // 128               # 16 contraction chunks for the projections
SCALE = 1.0 / float(np.sqrt(HD))

BF16 = ml_dtypes.bfloat16

_CACHE = {}


def _emit_body(nc, tc, pools, io):
    """Emit one full forward pass, batch-pipelined."""
    import concourse.bass as bass  # noqa: F401
    import concourse.mybir as mybir

    dt = mybir.dt
    (consts, xpool, big, temps, ntemps, probsp, outp, psum, psum_o,
     psum_l) = pools
    (xt_d, wq_d, wk_d, wv_d, wo_d, cs_d, sn_d, pt_d, onec_d, oner_d,
     msk_d, out_d) = io

    # ---- resident constants / weights in SBUF ----
    wq_sb = consts.tile([128, KC, 256], dt.bfloat16, tag="wq")
    wk_sb = consts.tile([128, KC, 256], dt.bfloat16, tag="wk")
    wv_sb = consts.tile([128, KC, 256], dt.bfloat16, tag="wv")
    nc.sync.dma_start(out=wq_sb, in_=wq_d.rearrange("(c p) e -> p c e", p=128))
    cs_sb = consts.tile([128, T], dt.bfloat16, tag="cs")
    sn_sb = consts.tile([128, T], dt.bfloat16, tag="sn")
    nc.sync.dma_start(out=cs_sb, in_=cs_d[:])
    nc.sync.dma_start(out=sn_sb, in_=sn_d[:])
    pt_sb = consts.tile([128, 128], dt.bfloat16, tag="pt")
    nc.sync.dma_start(out=pt_sb, in_=pt_d[:])
    onec_sb = consts.tile([128, 1], dt.bfloat16, tag="onec")
    nc.sync.dma_start(out=onec_sb, in_=onec_d[:])
    oner_sb = consts.tile([1, 128], dt.float32, tag="oner")
    nc.sync.dma_start(out=oner_sb, in_=oner_d[:])
    wo_sb = consts.tile([128, HPC, D], dt.bfloat16, tag="wo")
    msk_sb = consts.tile([128, 4, 512], dt.bfloat16, tag="msk")

    # ---- resident activations: per-(head,batch) for fine-grained deps ----
    qTs = {(h, b): big.tile([128, T], dt.bfloat16, tag=f"qT{h}{b}", name=f"qT{h}{b}")
           for h in range(HPC) for b in range(B)}
    kTs = {(h, b): big.tile([128, T], dt.bfloat16, tag=f"kT{h}{b}", name=f"kT{h}{b}")
           for h in range(HPC) for b in range(B)}
    oTs = {(h, b): big.tile([128, T], dt.bfloat16, tag=f"oT{h}{b}", name=f"oT{h}{b}")
           for h in range(HPC) for b in range(B)}
    vss = {b: big.tile([128, 16, 256], dt.bfloat16, tag=f"v{b}", name=f"v{b}")
           for b in range(B)}

    xt_r = xt_d.rearrange("(c p) t -> p c t", p=128)  # [128, 16, 4096]

    def late_consts():
        nc.sync.dma_start(out=wk_sb, in_=wk_d.rearrange("(c p) e -> p c e", p=128))
        nc.sync.dma_start(out=wv_sb, in_=wv_d.rearrange("(c p) e -> p c e", p=128))

    def proj_tile(b, ttl):
        t0g = (b * 4 + ttl) * 512   # global token offset
        t0 = ttl * 512              # within-batch offset
        xts = []
        for xh in range(2):
            xt_sb = xpool.tile([128, KC // 2, 512], dt.bfloat16, tag=f"xt{xh}",
                               name=f"xt{xh}")
            nc.sync.dma_start(
                out=xt_sb, in_=xt_r[:, xh * 8:(xh + 1) * 8, t0g:t0g + 512])
            xts.append(xt_sb)
        if b == 0 and ttl == 0:
            late_consts()

        for w_sb, dsts in ((wq_sb, qTs), (wk_sb, kTs)):
            for m in range(HPC):
                ps = psum.tile([128, 512], dt.float32, tag="s")
                for kc in range(KC):
                    nc.tensor.matmul(
                        ps,
                        lhsT=w_sb[:, kc, m * 128:(m + 1) * 128],
                        rhs=xts[kc // 8][:, kc % 8, :],
                        start=(kc == 0), stop=(kc == KC - 1),
                    )
                raw = temps.tile([128, 512], dt.bfloat16, tag="raw")
                nc.scalar.copy(out=raw, in_=ps)
                psr = psum.tile([128, 512], dt.float32, tag="s")
                nc.tensor.matmul(psr, lhsT=pt_sb, rhs=raw, start=True, stop=True)
                rsb = temps.tile([128, 512], dt.bfloat16, tag="rsb")
                nc.scalar.copy(out=rsb, in_=psr)
                t1 = temps.tile([128, 512], dt.bfloat16, tag="t1")
                nc.vector.tensor_tensor(t1, raw, cs_sb[:, t0:t0 + 512],
                                        mybir.AluOpType.mult)
                t2 = temps.tile([128, 512], dt.bfloat16, tag="t2")
                nc.vector.tensor_tensor(t2, rsb, sn_sb[:, t0:t0 + 512],
                                        mybir.AluOpType.mult)
                nc.gpsimd.tensor_tensor(dsts[(m, b)][:, t0:t0 + 512], t1, t2,
                                        mybir.AluOpType.add)

        # V -> [tok, hd]; two 256-wide groups share one psum bank
        for pair in range(2):
            psv = psum.tile([128, 512], dt.float32, tag="s")
            for half in range(2):
                sub = pair * 2 + half
                for kc in range(KC):
                    nc.tensor.matmul(
                        psv[:, half * 256:(half + 1) * 256],
                        lhsT=xts[kc // 8][:, kc % 8, sub * 128:(sub + 1) * 128],
                        rhs=wv_sb[:, kc, :],
                        start=(kc == 0 and half == 0),
                        stop=(kc == KC - 1),
                        skip_group_check=(half == 1),
                    )
            nc.scalar.copy(
                out=vss[b][:, ttl * 4 + pair * 2: ttl * 4 + pair * 2 + 2, :],
                in_=psv)

    def sdpa_block(b, qb, h, inter=None):
        q0 = qb * 512
        nk = 4 * (qb + 1)
        steps = 0.0
        ps_o = psum_o.tile([128, 512], dt.float32, tag="acc_o")
        ps_l = psum_l.tile([1, 512], dt.float32, tag="acc_l")
        for kc in range(nk):
            ps_s = psum.tile([128, 512], dt.float32, tag="s")
            nc.tensor.matmul(
                ps_s,
                lhsT=kTs[(h, b)][:, kc * 128:(kc + 1) * 128],
                rhs=qTs[(h, b)][:, q0:q0 + 512],
                start=True, stop=True,
            )
            pr = probsp.tile([128, 512], dt.bfloat16, tag="probs")
            nc.scalar.activation(pr, ps_s, mybir.ActivationFunctionType.Exp,
                                 scale=SCALE)
            moff = kc - 4 * qb
            if moff >= 0:
                nc.gpsimd.tensor_tensor(pr, pr, msk_sb[:, moff, :],
                                        mybir.AluOpType.mult)
            nc.tensor.matmul(ps_l, lhsT=onec_sb, rhs=pr,
                             start=(kc == 0), stop=(kc == nk - 1))
            nc.tensor.matmul(
                ps_o,
                lhsT=vss[b][:, kc, h * 128:(h + 1) * 128],
                rhs=pr,
                start=(kc == 0), stop=(kc == nk - 1),
            )
            if inter is not None:
                steps += 16.0 / nk
                while steps >= 1.0:
                    next(inter, None)
                    steps -= 1.0
        # normalize: oT = ps_o * recip(broadcast(l))
        l_row = ntemps.tile([1, 512], dt.float32, tag="l_row")
        nc.scalar.copy(out=l_row, in_=ps_l)
        ps_b = psum.tile([128, 512], dt.float32, tag="s")
        nc.tensor.matmul(ps_b, lhsT=oner_sb, rhs=l_row, start=True, stop=True)
        rb = ntemps.tile([128, 512], dt.float32, tag="rb")
        nc.vector.reciprocal(rb, ps_b)
        nc.vector.tensor_tensor(oTs[(h, b)][:, q0:q0 + 512], ps_o, rb,
                                mybir.AluOpType.mult)

    def outproj_gen(b, qb):
        q0 = qb * 512
        t0g = b * T + q0
        for dtile in range(D // 128):
            ps = psum.tile([128, 512], dt.float32, tag="s", name="ps_op")
            for h in range(HPC):
                nc.tensor.matmul(
                    ps,
                    lhsT=wo_sb[:, h, dtile * 128:(dtile + 1) * 128],
                    rhs=oTs[(h, b)][:, q0:q0 + 512],
                    start=(h == 0), stop=(h == HPC - 1),
                )
            o_sb = outp.tile([128, 512], dt.float32, tag="o_sb", name="o_sb")
            if dtile % 2 == 0:
                nc.scalar.copy(out=o_sb, in_=ps)
            else:
                nc.vector.tensor_copy(out=o_sb, in_=ps)
            nc.sync.dma_start(
                out=out_d[dtile * 128:(dtile + 1) * 128, t0g:t0g + 512],
                in_=o_sb)
            yield

    pending = None  # outproj runs one q-block behind SDPA to hide norm latency
    for b in range(B):
        for ttl in range(4):
            proj_tile(b, ttl)
            if b == 0 and ttl == 0:
                # late consts (not needed until sdpa/outproj)
                nc.sync.dma_start(
                    out=wo_sb, in_=wo_d.rearrange("(h p) e -> p h e", p=128))
                nc.sync.dma_start(out=msk_sb,
                                  in_=msk_d.rearrange("m p t -> p m t"))
        for qb in range(4):
            inter = outproj_gen(*pending) if pending is not None else None
            sdpa_block(b, qb, 0)
            sdpa_block(b, qb, 1, inter=inter)
            if inter is not None:
                for _ in inter:
                    pass
            pending = (b, qb)
    for _ in outproj_gen(*pending):
        pass


def build_nc(reps=1):
    """Build the Bass module. reps>1 wraps the body in a For_i loop executing
    it that many times (used only for wall-clock timing measurements)."""
    import concourse.bass as bass
    import concourse.mybir as mybir
    import concourse.tile as tile

    dt = mybir.dt
    nc = bass.Bass("TRN2", target_bir_lowering=False, debug=False,
                   num_devices=N_CORES)

    xt_d = nc.dram_tensor("xt", [D, TOK], dt.bfloat16, kind="ExternalInput")
    wq_d = nc.dram_tensor("wq", [D, 256], dt.bfloat16, kind="ExternalInput")
    wk_d = nc.dram_tensor("wk", [D, 256], dt.bfloat16, kind="ExternalInput")
    wv_d = nc.dram_tensor("wv", [D, 256], dt.bfloat16, kind="ExternalInput")
    wo_d = nc.dram_tensor("wo", [256, D], dt.bfloat16, kind="ExternalInput")
    cs_d = nc.dram_tensor("cs", [128, T], dt.bfloat16, kind="ExternalInput")
    sn_d = nc.dram_tensor("sn", [128, T], dt.bfloat16, kind="ExternalInput")
    pt_d = nc.dram_tensor("pt", [128, 128], dt.bfloat16, kind="ExternalInput")
    onec_d = nc.dram_tensor("onec", [128, 1], dt.bfloat16, kind="ExternalInput")
    oner_d = nc.dram_tensor("oner", [1, 128], dt.float32, kind="ExternalInput")
    msk_d = nc.dram_tensor("msk", [4, 128, 512], dt.bfloat16, kind="ExternalInput")
    out_d = nc.dram_tensor("out", [D, TOK], dt.float32, kind="ExternalOutput")
    io = (xt_d, wq_d, wk_d, wv_d, wo_d, cs_d, sn_d, pt_d, onec_d, oner_d,
          msk_d, out_d)

    with tile.TileContext(nc) as tc:
        import contextlib
        with contextlib.ExitStack() as ctx:
            consts = ctx.enter_context(tc.tile_pool(name="consts", bufs=1))
            xpool = ctx.enter_context(tc.tile_pool(name="xpool", bufs=2))
            big = ctx.enter_context(tc.tile_pool(name="big", bufs=1))
            temps = ctx.enter_context(tc.tile_pool(name="temps", bufs=4))
            ntemps = ctx.enter_context(tc.tile_pool(name="ntemps", bufs=2))
            probsp = ctx.enter_context(tc.tile_pool(name="probs", bufs=3))
            outp = ctx.enter_context(tc.tile_pool(name="outp", bufs=3))
            psum = ctx.enter_context(tc.tile_pool(name="psum", bufs=4,
                                                  space="PSUM"))
            psum_o = ctx.enter_context(tc.tile_pool(name="psum_o", bufs=2,
                                                    space="PSUM"))
            psum_l = ctx.enter_context(tc.tile_pool(name="psum_l", bufs=2,
                                                    space="PSUM"))
            pools = (consts, xpool, big, temps, ntemps, probsp, outp, psum,
                     psum_o, psum_l)
            if reps > 1:
                with tc.For_i(0, reps, 1):
                    _emit_body(nc, tc, pools, io)
            else:
                _emit_body(nc, tc, pools, io)

    return nc


def _split_sync_waits(nc, max_waits=1):
    """Walrus in this env rejects instructions with too many sync waits.
    Hoist excess waits onto preceding same-engine nops."""
    import bass_rust
    import concourse.mybir as mybir

    n_split = 0
    for f in nc.m.functions:
        for bb in f.blocks:
            insts = bb.instructions
            new = []
            dirty = False
            for inst in insts:
                si = inst.sync_info
                if si is not None and si.on_wait and len(si.on_wait) > max_waits:
                    waits = list(si.on_wait)
                    for j, w in enumerate(waits[:-max_waits]):
                        n = mybir.InstNoOp(name=f"{inst.name}-wsplit{j}",
                                           ins=[], outs=[])
                        n.engine = inst.engine
                        n.sync_info = bass_rust.SyncInfo(on_wait=[w], on_update=[])
                        new.append(n)
                        n_split += 1
                    si.on_wait = waits[-max_waits:]
                    inst.sync_info = si
                    dirty = True
                new.append(inst)
            if dirty:
                bb.instructions = new
    return n_split


def _host_prep(x, cos, sin, Wqkv, Wout):
    """Shard + lay out inputs for each core. Returns in_maps list."""
    xf = np.ascontiguousarray(x.reshape(TOK, D).T).astype(BF16)        # [D, TOK]
    csT = np.ascontiguousarray(cos.T)                                   # [128, T]
    snT = np.ascontiguousarray(sin.T)
    cs4 = csT.astype(BF16)                                              # [128, T]
    sn4 = snT.astype(BF16)

    # rotate_half permutation: rot = P @ u ; pt = P.T
    P = np.zeros((128, 128), dtype=np.float32)
    P[np.arange(64), np.arange(64) + 64] = -1.0
    P[np.arange(64) + 64, np.arange(64)] = 1.0
    pt = np.ascontiguousarray(P.T).astype(BF16)

    onec = np.ones((128, 1), dtype=np.float32).astype(BF16)
    oner = np.ones((1, 128), dtype=np.float32)

    msk = np.zeros((4, 128, 512), dtype=np.float32)
    for m in range(4):
        off = m * 128
        kk = np.arange(128)[:, None]
        qq = np.arange(512)[None, :]
        msk[m] = (off + kk <= qq).astype(np.float32)
    msk = msk.astype(BF16)

    in_maps = []
    for c in range(N_CORES):
        r0 = c * HPC * HD
        r1 = (c + 1) * HPC * HD
        wq = np.ascontiguousarray(Wqkv[r0:r1, :].T).astype(BF16)          # [D, 256]
        wk = np.ascontiguousarray(Wqkv[D + r0:D + r1, :].T).astype(BF16)
        wv = np.ascontiguousarray(Wqkv[2 * D + r0:2 * D + r1, :].T).astype(BF16)
        wo = np.ascontiguousarray(Wout[:, r0:r1].T).astype(BF16)          # [256, D]
        in_maps.append({
            "xt": xf, "wq": wq, "wk": wk, "wv": wv, "wo": wo,
            "cs": cs4, "sn": sn4, "pt": pt, "onec": onec, "oner": oner,
            "msk": msk,
        })
    return in_maps


def kernel(x, cos, sin, Wqkv, Wout):
    from concourse.bass_utils import run_bass_kernel_spmd

    x = np.asarray(x, dtype=np.float32)
    cos = np.asarray(cos, dtype=np.float32)
    sin = np.asarray(sin, dtype=np.float32)
    Wqkv = np.asarray(Wqkv, dtype=np.float32)
    Wout = np.asarray(Wout, dtype=np.float32)

    if "nc" not in _CACHE:
        nc = build_nc()
        _split_sync_waits(nc, max_waits=1)
        _CACHE["nc"] = nc
    nc = _CACHE["nc"]

    in_maps = _host_prep(x, cos, sin, Wqkv, Wout)
    res = run_bass_kernel_spmd(nc, in_maps, core_ids=list(range(N_CORES)))
    acc = np.zeros((D, TOK), dtype=np.float32)
    for c in range(N_CORES):
        acc += res.results[c]["out"]
    return np.ascontiguousarray(acc.T).reshape(B, T, D)
